# revision 2
# baseline (speedup 1.0000x reference)
"""SwiGLU FFN (gate/up/down) on 8 TRN2 NeuronCores.

Strategy: data-parallel over tokens. Each core gets 1024 tokens and the
full weight set. All matmuls run in bf16 with fp32 PSUM accumulation.

Layout trick: activations are kept transposed on-chip (feature dim on
partitions, tokens on the free dim), so every matmul has its contraction
dim on partitions for both operands and no on-device transposes are
needed:
  gate.T = Wg_lhsT.T @ x.T    (lhsT[k,m] = Wg[m,k], k = hidden)
  h.T    = silu(gate.T) * up.T
  y.T    = Wd_lhsT.T @ h.T    (lhsT[k,m] = Wd[m,k], k = inter)

Weights are pre-tiled on the host into [m_tile, p, (g), k_tile, m] order
so each per-m-tile DMA reads 16-22KB contiguous per partition.

SBUF budget per partition (of ~208KB usable): xT 32KB + hT 86KB +
weight slots 2x22KB + staging ~12KB.
"""

import numpy as np
import ml_dtypes

import concourse.bass as bass
import concourse.tile as tile
import concourse.mybir as mybir
from concourse.bass_utils import run_bass_kernel_spmd

BF16 = ml_dtypes.bfloat16

P = 128
HID = 4096
INT = 11008
TOK = 8192
NCORES = 8
TPC = TOK // NCORES          # tokens per core
T = 512                      # tokens per pass (PSUM free-dim limit, f32)
NPASS = TPC // T
KTH = HID // P               # 32 k-tiles over hidden
MTI = INT // P               # 86 m-tiles over intermediate
MTH = HID // P               # 32 m-tiles over hidden (down proj)
KTI = INT // P               # 86 k-tiles over intermediate


def _split_multiwaits(nc):
    # This walrus build supports a single sync-wait slot per instruction;
    # hoist extra waits onto single-wait NoOps inserted just before the
    # offending instruction on the same engine (same semantics: the engine
    # stream blocks on each wait in order).
    n = 0
    for f in nc.m.functions:
        for blk in f.blocks:
            insts = blk.instructions  # live list
            i = 0
            while i < len(insts):
                inst = insts[i]
                si = getattr(inst, "sync_info", None)
                if si is not None and si.on_wait and len(si.on_wait) > 1:
                    waits = list(si.on_wait)
                    for j, w in enumerate(waits[:-1]):
                        nop = mybir.InstNoOp(
                            name=f"{inst.name}_splitwait{j}", ins=[], outs=[]
                        )
                        nop.engine = inst.engine
                        nop.sync_info = mybir.SyncInfo(on_wait=[w], on_update=[])
                        insts.insert(i, nop)
                        i += 1
                        n += 1
                    inst.sync_info = mybir.SyncInfo(
                        on_wait=[waits[-1]], on_update=list(si.on_update)
                    )
                i += 1
    return n


def build_nc():
    bf = mybir.dt.bfloat16
    f32 = mybir.dt.float32
    nc = bass.Bass()

    xt = nc.dram_tensor("xt", [NPASS, P, KTH, T], bf, kind="ExternalInput")
    wgu = nc.dram_tensor("wgu", [MTI, P, 2, KTH, P], bf, kind="ExternalInput")
    wd = nc.dram_tensor("wd", [MTH, P, KTI, P], bf, kind="ExternalInput")
    yt = nc.dram_tensor("yt", [NPASS, MTH, P, T], f32, kind="ExternalOutput")

    with tile.TileContext(nc) as tc:
        with (
            tc.tile_pool(name="xp", bufs=1) as xp,
            tc.tile_pool(name="hp", bufs=1) as hp,
            tc.tile_pool(name="wp", bufs=2) as wp,
            tc.tile_pool(name="sp", bufs=3) as sp,
            tc.tile_pool(name="pg", bufs=2, space="PSUM") as pg,
            tc.tile_pool(name="py", bufs=2, space="PSUM") as py,
        ):
            for ps in range(NPASS):
                xt_sb = xp.tile([P, KTH, T], bf, name="xt_sb", tag="xt_sb")
                nc.sync.dma_start(xt_sb[:], xt[ps])
                ht = hp.tile([P, MTI, T], bf, name="ht", tag="ht")
                for mt in range(MTI):
                    w = wp.tile([P, 2, KTH, P], bf, name="w_gu", tag="w")
                    nc.sync.dma_start(w[:], wgu[mt])
                    g_ps = pg.tile([P, T], f32, name="g_ps", tag="g")
                    u_ps = pg.tile([P, T], f32, name="u_ps", tag="u")
                    for kt in range(KTH):
                        nc.tensor.matmul(
                            g_ps[:], w[:, 0, kt], xt_sb[:, kt],
                            start=(kt == 0), stop=(kt == KTH - 1),
                        )
                    for kt in range(KTH):
                        nc.tensor.matmul(
                            u_ps[:], w[:, 1, kt], xt_sb[:, kt],
                            start=(kt == 0), stop=(kt == KTH - 1),
                        )
                    sil = sp.tile([P, T], f32, name="sil", tag="sil")
                    nc.scalar.activation(
                        sil[:], g_ps[:], mybir.ActivationFunctionType.Silu
                    )
                    nc.vector.tensor_mul(ht[:, mt], sil[:], u_ps[:])
                for mh in range(MTH):
                    wdt = wp.tile([P, KTI, P], bf, name="w_d", tag="w")
                    nc.sync.dma_start(wdt[:], wd[mh])
                    y_ps = py.tile([P, T], f32, name="y_ps", tag="y")
                    for kt in range(KTI):
                        nc.tensor.matmul(
                            y_ps[:], wdt[:, kt], ht[:, kt],
                            start=(kt == 0), stop=(kt == KTI - 1),
                        )
                    y_sb = sp.tile([P, T], f32, name="y_sb", tag="ysb")
                    nc.vector.tensor_copy(y_sb[:], y_ps[:])
                    nc.sync.dma_start(yt[ps, mh], y_sb[:])

    _split_multiwaits(nc)
    return nc


def prep_inputs(x, W_gate, W_up, W_down):
    # lhsT layouts: element [mt, p, (g,) kt, m] = W[mt*128+m, kt*128+p]
    wg = W_gate.reshape(MTI, P, KTH, P).transpose(0, 3, 2, 1)
    wu = W_up.reshape(MTI, P, KTH, P).transpose(0, 3, 2, 1)
    wgu = np.stack([wg, wu], axis=2).astype(BF16)          # [mt, p, 2, kt, m]
    wd = W_down.reshape(MTH, P, KTI, P).transpose(0, 3, 2, 1).astype(BF16)
    # x: [core, pass, t, kt, p] -> per-core [pass, p, kt, t]
    xr = x.reshape(NCORES, NPASS, T, KTH, P)
    xts = [np.ascontiguousarray(xr[c].transpose(0, 3, 2, 1)).astype(BF16)
           for c in range(NCORES)]
    return xts, wgu, wd


_NC_CACHE = []


def get_nc():
    if not _NC_CACHE:
        _NC_CACHE.append(build_nc())
    return _NC_CACHE[0]


def make_in_maps(inputs):
    xts, wgu, wd = prep_inputs(
        np.asarray(inputs["x"], np.float32),
        np.asarray(inputs["W_gate"], np.float32),
        np.asarray(inputs["W_up"], np.float32),
        np.asarray(inputs["W_down"], np.float32),
    )
    return [{"xt": xts[c], "wgu": wgu, "wd": wd} for c in range(NCORES)]


def kernel(x, W_gate, W_up, W_down):
    x = np.asarray(x, dtype=np.float32)
    xts, wgu, wd = prep_inputs(
        np.asarray(x, np.float32),
        np.asarray(W_gate, np.float32),
        np.asarray(W_up, np.float32),
        np.asarray(W_down, np.float32),
    )
    nc = get_nc()
    in_maps = [{"xt": xts[c], "wgu": wgu, "wd": wd} for c in range(NCORES)]
    res = run_bass_kernel_spmd(nc, in_maps, core_ids=list(range(NCORES)))
    out = np.empty((TOK, HID), np.float32)
    for c in range(NCORES):
        ytc = res.results[c]["yt"]                          # [pass, mh, p, t]
        out[c * TPC:(c + 1) * TPC] = (
            ytc.transpose(0, 3, 1, 2).reshape(TPC, HID)
        )
    return out



# revision 6
# speedup vs baseline: 1.0670x; 1.0670x over previous
"""SwiGLU FFN (gate/up/down) on 8 TRN2 NeuronCores.

Strategy: data-parallel over tokens. Each core gets 1024 tokens (as two
512-token passes) and the full weight set. All matmuls run in bf16 with
fp32 PSUM accumulation.

v2: weights are read from HBM exactly once per execution. The gate/up
loop visits each weight m-tile once and computes BOTH 512-token passes
while the tile is resident (the baseline re-read all 260MB of weights per
pass; with 8 cores streaming 4.2GB aggregate the cores contend for HBM
and the PE stalls). The intermediate activation h (11008 x 1024 bf16,
too large for SBUF together with the inputs) is staged through a DRAM
scratch buffer: written per m-tile during gate/up, read back as one
resident SBUF tile for the down projection.

Layout trick: activations are kept transposed on-chip (feature dim on
partitions, tokens on the free dim), so every matmul has its contraction
dim on partitions for both operands and no on-device transposes are
needed:
  gate.T = Wg_lhsT.T @ x.T    (lhsT[k,m] = Wg[m,k], k = hidden)
  h.T    = silu(gate.T) * up.T
  y.T    = Wd_lhsT.T @ h.T    (lhsT[k,m] = Wd[m,k], k = inter)
"""

import numpy as np
import ml_dtypes

import concourse.bass as bass
import concourse.tile as tile
import concourse.mybir as mybir
from concourse.bass_utils import run_bass_kernel_spmd

BF16 = ml_dtypes.bfloat16

P = 128
HID = 4096
INT = 11008
TOK = 8192
NCORES = 8
TPC = TOK // NCORES          # tokens per core
T = 512                      # tokens per pass (PSUM free-dim limit, f32)
NP = TPC // T                # token passes (2)
KTH = HID // P               # 32 k-tiles over hidden
MTI = INT // P               # 86 m-tiles over intermediate
MTH = HID // P               # 32 m-tiles over hidden (down proj)
KTI = INT // P               # 86 k-tiles over intermediate

# wd streamed in 6 sub-chunks per m-tile to keep the SBUF slot small
# while h (172KB/partition) is resident
WD_CH = [15, 15, 14, 14, 14, 14]
WD_OFF = [0, 15, 30, 44, 58, 72]
# h read back in chunks so the down phase can prefetch while gate/up
# is still finishing
HRD = 43


def _split_multiwaits(nc):
    # This walrus build supports a single sync-wait slot per instruction;
    # hoist extra waits onto single-wait NoOps inserted just before the
    # offending instruction on the same engine (same semantics: the engine
    # stream blocks on each wait in order).
    n = 0
    for f in nc.m.functions:
        for blk in f.blocks:
            insts = blk.instructions  # live list
            i = 0
            while i < len(insts):
                inst = insts[i]
                si = getattr(inst, "sync_info", None)
                if si is not None and si.on_wait and len(si.on_wait) > 1:
                    waits = list(si.on_wait)
                    for j, w in enumerate(waits[:-1]):
                        nop = mybir.InstNoOp(
                            name=f"{inst.name}_splitwait{j}", ins=[], outs=[]
                        )
                        nop.engine = inst.engine
                        nop.sync_info = mybir.SyncInfo(on_wait=[w], on_update=[])
                        insts.insert(i, nop)
                        i += 1
                        n += 1
                    inst.sync_info = mybir.SyncInfo(
                        on_wait=[waits[-1]], on_update=list(si.on_update)
                    )
                i += 1
    return n


def build_nc():
    bf = mybir.dt.bfloat16
    f32 = mybir.dt.float32
    nc = bass.Bass()

    xt = nc.dram_tensor("xt", [P, KTH, NP, T], bf, kind="ExternalInput")
    wgu = nc.dram_tensor("wgu", [MTI, P, 2, KTH, P], bf, kind="ExternalInput")
    wd = nc.dram_tensor("wd", [MTH, P, KTI, P], bf, kind="ExternalInput")
    yt = nc.dram_tensor("yt", [NP, MTH, P, T], f32, kind="ExternalOutput")

    with tile.TileContext(nc) as tc:
        with tc.tile_pool(name="hd", bufs=1, space="DRAM") as hd:
            h_dram = hd.tile([NP, P, MTI, T], bf, name="h_dram")

            # ---- phase G: gate/up, one visit per weight m-tile ----
            with (
                tc.tile_pool(name="xp", bufs=1) as xp,
                tc.tile_pool(name="wp", bufs=2) as wp,
                tc.tile_pool(name="sp", bufs=2) as sp,
                tc.tile_pool(name="hs", bufs=4) as hs,
                tc.tile_pool(name="pg", bufs=1, space="PSUM") as pg,
            ):
                xt_sb = xp.tile([P, KTH, NP, T], bf, name="xt_sb")
                nc.sync.dma_start(xt_sb[:], xt[:])
                for mt in range(MTI):
                    w = wp.tile([P, 2, KTH, P], bf, name="w_gu", tag="w")
                    nc.sync.dma_start(w[:], wgu[mt])
                    gu = [
                        [pg.tile([P, T], f32, name=f"ps_{g}{p}", tag=f"ps{g}{p}")
                         for p in range(NP)]
                        for g in range(2)
                    ]
                    for g in range(2):
                        for kt in range(KTH):
                            for p in range(NP):
                                nc.tensor.matmul(
                                    gu[g][p][:], w[:, g, kt], xt_sb[:, kt, p],
                                    start=(kt == 0), stop=(kt == KTH - 1),
                                )
                    for p in range(NP):
                        sil = sp.tile([P, T], f32, name="sil", tag="sil")
                        nc.scalar.activation(
                            sil[:], gu[0][p][:], mybir.ActivationFunctionType.Silu
                        )
                        hst = hs.tile([P, T], bf, name="hst", tag="hst")
                        nc.vector.tensor_mul(hst[:], sil[:], gu[1][p][:])
                        nc.sync.dma_start(h_dram[p, :, mt, :], hst[:])

            # ---- phase D: down projection, h resident ----
            with (
                tc.tile_pool(name="hp", bufs=1) as hp,
                tc.tile_pool(name="wdp", bufs=2) as wdp,
                tc.tile_pool(name="yp", bufs=3) as yp,
                tc.tile_pool(name="py", bufs=2, space="PSUM") as py,
            ):
                ht = hp.tile([P, NP, MTI, T], bf, name="ht")
                for p in range(NP):
                    for m0 in range(0, MTI, HRD):
                        m1 = min(m0 + HRD, MTI)
                        nc.sync.dma_start(
                            ht[:, p, m0:m1, :], h_dram[p, :, m0:m1, :]
                        )
                for mh in range(MTH):
                    wdc = [
                        wdp.tile([P, WD_CH[c], P], bf, name=f"wd{c}", tag="wd")
                        for c in range(6)
                    ]
                    for c in range(6):
                        o = WD_OFF[c]
                        nc.sync.dma_start(
                            wdc[c][:], wd[mh][:, o:o + WD_CH[c]]
                        )
                    ys = [py.tile([P, T], f32, name=f"y_ps{p}", tag=f"y{p}")
                          for p in range(NP)]
                    c = 0
                    for kt in range(KTI):
                        if kt >= WD_OFF[c] + WD_CH[c]:
                            c += 1
                        kl = kt - WD_OFF[c]
                        for p in range(NP):
                            nc.tensor.matmul(
                                ys[p][:], wdc[c][:, kl], ht[:, p, kt],
                                start=(kt == 0), stop=(kt == KTI - 1),
                            )
                    for p in range(NP):
                        y_sb = yp.tile([P, T], f32, name="y_sb", tag="ysb")
                        nc.vector.tensor_copy(y_sb[:], ys[p][:])
                        nc.sync.dma_start(yt[p, mh], y_sb[:])

    _split_multiwaits(nc)
    return nc


def prep_inputs(x, W_gate, W_up, W_down):
    # lhsT layouts: element [mt, p, (g,) kt, m] = W[mt*128+m, kt*128+p]
    wg = W_gate.reshape(MTI, P, KTH, P).transpose(0, 3, 2, 1)
    wu = W_up.reshape(MTI, P, KTH, P).transpose(0, 3, 2, 1)
    wgu = np.stack([wg, wu], axis=2).astype(BF16)          # [mt, p, 2, kt, m]
    wd = W_down.reshape(MTH, P, KTI, P).transpose(0, 3, 2, 1).astype(BF16)
    # x: [core, np, t, kt, p] -> per-core [p, kt, np, t]
    xr = x.reshape(NCORES, NP, T, KTH, P)
    xts = [np.ascontiguousarray(xr[c].transpose(3, 2, 0, 1)).astype(BF16)
           for c in range(NCORES)]
    return xts, wgu, wd


_NC_CACHE = []


def get_nc():
    if not _NC_CACHE:
        _NC_CACHE.append(build_nc())
    return _NC_CACHE[0]


def make_in_maps(inputs):
    xts, wgu, wd = prep_inputs(
        np.asarray(inputs["x"], np.float32),
        np.asarray(inputs["W_gate"], np.float32),
        np.asarray(inputs["W_up"], np.float32),
        np.asarray(inputs["W_down"], np.float32),
    )
    return [{"xt": xts[c], "wgu": wgu, "wd": wd} for c in range(NCORES)]


def kernel(x, W_gate, W_up, W_down):
    x = np.asarray(x, dtype=np.float32)
    xts, wgu, wd = prep_inputs(
        np.asarray(x, np.float32),
        np.asarray(W_gate, np.float32),
        np.asarray(W_up, np.float32),
        np.asarray(W_down, np.float32),
    )
    nc = get_nc()
    in_maps = [{"xt": xts[c], "wgu": wgu, "wd": wd} for c in range(NCORES)]
    res = run_bass_kernel_spmd(nc, in_maps, core_ids=list(range(NCORES)))
    out = np.empty((TOK, HID), np.float32)
    for c in range(NCORES):
        ytc = res.results[c]["yt"]                          # [np, mh, p, t]
        out[c * TPC:(c + 1) * TPC] = (
            ytc.transpose(0, 3, 1, 2).reshape(TPC, HID)
        )
    return out


# revision 7
# speedup vs baseline: 1.1768x; 1.1028x over previous
"""SwiGLU FFN (gate/up/down) on 8 TRN2 NeuronCores.

Strategy: data-parallel over tokens. Each core gets 1024 tokens and the
full weight set. All matmuls run in bf16 with fp32 PSUM accumulation.

Layout trick: activations are kept transposed on-chip (feature dim on
partitions, tokens on the free dim), so every matmul has its contraction
dim on partitions for both operands and no on-device transposes are
needed:
  gate.T = Wg_lhsT.T @ x.T    (lhsT[k,m] = Wg[m,k], k = hidden)
  h.T    = silu(gate.T) * up.T
  y.T    = Wd_lhsT.T @ h.T    (lhsT[k,m] = Wd[m,k], k = inter)

Weights are pre-tiled on the host into [m_tile, p, (g), k_tile, m] order
so each per-m-tile DMA reads 16-22KB contiguous per partition.

SBUF budget per partition (of ~208KB usable): xT 32KB + hT 86KB +
weight slots 2x22KB + staging ~12KB.
"""

import numpy as np
import ml_dtypes

import concourse.bass as bass
import concourse.tile as tile
import concourse.mybir as mybir
from concourse.bass_utils import run_bass_kernel_spmd

BF16 = ml_dtypes.bfloat16

P = 128
HID = 4096
INT = 11008
TOK = 8192
NCORES = 8
TPC = TOK // NCORES          # tokens per core
T = 512                      # tokens per pass (PSUM free-dim limit, f32)
NPASS = TPC // T
KTH = HID // P               # 32 k-tiles over hidden
MTI = INT // P               # 86 m-tiles over intermediate
MTH = HID // P               # 32 m-tiles over hidden (down proj)
KTI = INT // P               # 86 k-tiles over intermediate


def _split_multiwaits(nc):
    # This walrus build supports a single sync-wait slot per instruction;
    # hoist extra waits onto single-wait NoOps inserted just before the
    # offending instruction on the same engine (same semantics: the engine
    # stream blocks on each wait in order).
    n = 0
    for f in nc.m.functions:
        for blk in f.blocks:
            insts = blk.instructions  # live list
            i = 0
            while i < len(insts):
                inst = insts[i]
                si = getattr(inst, "sync_info", None)
                if si is not None and si.on_wait and len(si.on_wait) > 1:
                    waits = list(si.on_wait)
                    for j, w in enumerate(waits[:-1]):
                        nop = mybir.InstNoOp(
                            name=f"{inst.name}_splitwait{j}", ins=[], outs=[]
                        )
                        nop.engine = inst.engine
                        nop.sync_info = mybir.SyncInfo(on_wait=[w], on_update=[])
                        insts.insert(i, nop)
                        i += 1
                        n += 1
                    inst.sync_info = mybir.SyncInfo(
                        on_wait=[waits[-1]], on_update=list(si.on_update)
                    )
                i += 1
    return n


def build_nc():
    bf = mybir.dt.bfloat16
    f32 = mybir.dt.float32
    nc = bass.Bass()

    xt = nc.dram_tensor("xt", [NPASS, P, KTH, T], bf, kind="ExternalInput")
    wgu = nc.dram_tensor("wgu", [MTI, P, 2, KTH, P], bf, kind="ExternalInput")
    wd = nc.dram_tensor("wd", [MTH, P, KTI, P], bf, kind="ExternalInput")
    yt = nc.dram_tensor("yt", [NPASS, MTH, P, T], f32, kind="ExternalOutput")

    with tile.TileContext(nc) as tc:
        with (
            tc.tile_pool(name="xp", bufs=1) as xp,
            tc.tile_pool(name="hp", bufs=1) as hp,
            tc.tile_pool(name="wp", bufs=2) as wp,
            tc.tile_pool(name="sp", bufs=3) as sp,
            tc.tile_pool(name="pg", bufs=2, space="PSUM") as pg,
            tc.tile_pool(name="py", bufs=2, space="PSUM") as py,
        ):
            for ps in range(NPASS):
                xt_sb = xp.tile([P, KTH, T], bf, name="xt_sb", tag="xt_sb")
                nc.sync.dma_start(xt_sb[:], xt[ps])
                ht = hp.tile([P, MTI, T], bf, name="ht", tag="ht")
                for mt in range(MTI):
                    w = wp.tile([P, 2, KTH, P], bf, name="w_gu", tag="w")
                    nc.sync.dma_start(w[:], wgu[mt])
                    g_ps = pg.tile([P, T], f32, name="g_ps", tag="g")
                    u_ps = pg.tile([P, T], f32, name="u_ps", tag="u")
                    for kt in range(KTH):
                        nc.tensor.matmul(
                            g_ps[:], w[:, 0, kt], xt_sb[:, kt],
                            start=(kt == 0), stop=(kt == KTH - 1),
                        )
                    for kt in range(KTH):
                        nc.tensor.matmul(
                            u_ps[:], w[:, 1, kt], xt_sb[:, kt],
                            start=(kt == 0), stop=(kt == KTH - 1),
                        )
                    sil = sp.tile([P, T], f32, name="sil", tag="sil")
                    nc.scalar.activation(
                        sil[:], g_ps[:], mybir.ActivationFunctionType.Silu
                    )
                    nc.vector.tensor_mul(ht[:, mt], sil[:], u_ps[:])
                for mh in range(MTH):
                    wdt = wp.tile([P, KTI, P], bf, name="w_d", tag="w")
                    nc.sync.dma_start(wdt[:], wd[mh])
                    y_ps = py.tile([P, T], f32, name="y_ps", tag="y")
                    for kt in range(KTI):
                        nc.tensor.matmul(
                            y_ps[:], wdt[:, kt], ht[:, kt],
                            start=(kt == 0), stop=(kt == KTI - 1),
                        )
                    y_sb = sp.tile([P, T], f32, name="y_sb", tag="ysb")
                    nc.vector.tensor_copy(y_sb[:], y_ps[:])
                    nc.sync.dma_start(yt[ps, mh], y_sb[:])

    _split_multiwaits(nc)
    return nc


def prep_inputs(x, W_gate, W_up, W_down):
    # lhsT layouts: element [mt, p, (g,) kt, m] = W[mt*128+m, kt*128+p]
    wg = W_gate.reshape(MTI, P, KTH, P).transpose(0, 3, 2, 1)
    wu = W_up.reshape(MTI, P, KTH, P).transpose(0, 3, 2, 1)
    wgu = np.stack([wg, wu], axis=2).astype(BF16)          # [mt, p, 2, kt, m]
    wd = W_down.reshape(MTH, P, KTI, P).transpose(0, 3, 2, 1).astype(BF16)
    # x: [core, pass, t, kt, p] -> per-core [pass, p, kt, t]
    xr = x.reshape(NCORES, NPASS, T, KTH, P)
    xts = [np.ascontiguousarray(xr[c].transpose(0, 3, 2, 1)).astype(BF16)
           for c in range(NCORES)]
    return xts, wgu, wd


_NC_CACHE = []


def get_nc():
    if not _NC_CACHE:
        _NC_CACHE.append(build_nc())
    return _NC_CACHE[0]


def make_in_maps(inputs):
    xts, wgu, wd = prep_inputs(
        np.asarray(inputs["x"], np.float32),
        np.asarray(inputs["W_gate"], np.float32),
        np.asarray(inputs["W_up"], np.float32),
        np.asarray(inputs["W_down"], np.float32),
    )
    return [{"xt": xts[c], "wgu": wgu, "wd": wd} for c in range(NCORES)]


def kernel(x, W_gate, W_up, W_down):
    x = np.asarray(x, dtype=np.float32)
    xts, wgu, wd = prep_inputs(
        np.asarray(x, np.float32),
        np.asarray(W_gate, np.float32),
        np.asarray(W_up, np.float32),
        np.asarray(W_down, np.float32),
    )
    nc = get_nc()
    in_maps = [{"xt": xts[c], "wgu": wgu, "wd": wd} for c in range(NCORES)]
    res = run_bass_kernel_spmd(nc, in_maps, core_ids=list(range(NCORES)))
    out = np.empty((TOK, HID), np.float32)
    for c in range(NCORES):
        ytc = res.results[c]["yt"]                          # [pass, mh, p, t]
        out[c * TPC:(c + 1) * TPC] = (
            ytc.transpose(0, 3, 1, 2).reshape(TPC, HID)
        )
    return out

